# revision 4
# baseline (speedup 1.0000x reference)
"""Trainium2 kernel for nn_MeanSquaredError2 (scatter_memory).

Math: the reference builds, per (batch, channel), a gaussian-filtered one-hot
target map tt, min-max normalizes it, masks by visibility, and returns
sum(mask*(h-tt)^2) / (v.sum()/2).

Factorization (validated to ~8e-6 rel err vs reference):
  sum mask*(h-tt)^2 = sum_vis h^2 - 2*sum_vis <h, tt> + sum_vis tt^2
The filtered one-hot of a pixel q=(y*14+x) is a fixed table row M1[q, :]
(196 values); with M1' = M1 - min(M1[q]) the per-channel target is
tt = (sum_m M1'[q_m] + r)/d with per-channel scalars r, d computed host-side
(joints: 1 pixel, r=0, d a function of q only; groups: up to 3 deduped
pixels).  The only h-coupled device work is:
  SJ[p, q] = sum_{joint rows} h[row, p] * onehot(q_row)[q]
  SG[p, q] = sum_{group rows} (1/d_row) * h[row, p] * multihot(row)[q]
  SG[p,196]= sum_{group rows} (r_row/d_row) * h[row, p]
  SQ       = sum_rows sum_p h[row, p]^2
Host finishes with <SJ, (M1'/d)^T>, <SG[:, :196], M1'^T>, sum SG[:, 196], SQ.
Invisible channels contribute nothing and are dropped host-side (~45% of h).

Device (SPMD over 8 cores, batch-sharded):
  - DMA visible rows (one 196-float row per visible channel) in [128, 4*196]
    super-tiles.
  - ScalarE: Square+accumulate per super-tile -> SQ slots.
  - VectorE: build one-hot weight vectors W[row, :] = (iota==q_row)[*wq]
    via single fused tensor_scalar ops.
  - TensorE: scatter accumulation in PSUM: out[pixel, q] += H_tile^T @ W_tile.
"""

import os as _os
import sys
import numpy as np

for _p in ("/opt/trn_rl_repo", "/root/.axon_site/_ro/trn_rl_repo"):
    if _p not in sys.path:
        sys.path.append(_p)

import concourse.bass as bass  # noqa: E402
import concourse.tile as tile  # noqa: E402
from concourse import mybir  # noqa: E402
from concourse.bass_utils import run_bass_kernel_spmd  # noqa: E402

COL = 14
NJ = 14
RADIUS = 4
B = 8192
NCORES = 8
BS = B // NCORES  # 1024 samples per core
SENT = 999.0  # sentinel pixel index: never matches iota 0..195

# staged row capacities per core (128-row tiles, 4-tile supers)
JTILES = 64  # joint tiles  -> capacity 8192 rows (expect ~7200)
GTILES = 32  # group tiles  -> capacity 4096 rows (theoretical max 4096)
JSUP = JTILES // 4  # 16
GSUP = GTILES // 4  # 8
JCAP = JTILES * 128
GCAP = GTILES * 128

F32 = mybir.dt.float32


# ---------------------------------------------------------------- host tables
_tables_cache = None


def _tables():
    global _tables_cache
    if _tables_cache is not None:
        return _tables_cache
    x = np.arange(-RADIUS, RADIUS + 1).astype(np.float32)
    k = np.exp(-0.5 * x * x)
    k = (k / k.sum()).astype(np.float32)
    Km = np.zeros((COL, COL), np.float32)
    for p in range(COL):
        v = np.zeros(COL, np.float32)
        v[p] = 1.0
        vp = np.pad(v, RADIUS, mode="symmetric")
        Km[:, p] = np.convolve(vp, k[::-1], mode="valid").astype(np.float32)
    M1 = np.zeros((196, 196), np.float32)
    for yi in range(COL):
        for xi in range(COL):
            M1[yi * COL + xi] = np.outer(Km[:, yi], Km[:, xi]).reshape(196)
    mn_q = M1.min(axis=1)
    d_q = M1.max(axis=1) - mn_q
    M1p = (M1 - mn_q[:, None]).astype(np.float64)
    T2j = ((M1p / d_q[:, None]) ** 2).sum(axis=1)
    M1ext = np.concatenate([M1, np.zeros((1, 196), np.float32)])
    mn_qext = np.concatenate([mn_q, [0.0]]).astype(np.float64)
    _tables_cache = (M1p, mn_q, d_q, T2j, M1ext, mn_qext)
    return _tables_cache


def _host_stage(h, t, v):
    """Build per-core staged rows + metadata, plus host-only scalar terms."""
    M1p, mn_q, d_q, T2j, M1ext, mn_qext = _tables()
    h = np.ascontiguousarray(h, dtype=np.float32).reshape(B, 18, 196)
    ti = t.astype(np.float32) * COL
    idx = np.clip(ti.astype(np.int32), 0, COL - 1)
    xi, yi = idx[..., 0], idx[..., 1]
    vis = v[..., 0] == 1  # [B, NJ]
    q = (yi.astype(np.int64) * COL + xi.astype(np.int64))  # [B, NJ]

    # ---- joints ----
    bj = np.argwhere(vis)
    bs, js = bj[:, 0], bj[:, 1]
    qj = q[bs, js]
    hostD = float(T2j[qj].sum())

    # ---- groups ----
    gvis = vis[:, :12].reshape(B, 4, 3).any(axis=2)
    bg = np.argwhere(gvis)
    gb, gg = bg[:, 0], bg[:, 1]
    n_g = len(gb)
    qm = np.full((n_g, 3), 196, np.int64)
    for m in range(3):
        jj = gg * 3 + m
        vism = vis[gb, jj]
        qmv = q[gb, jj]
        dup = np.zeros(n_g, bool)
        for m2 in range(m):
            jj2 = gg * 3 + m2
            dup |= vis[gb, jj2] & (q[gb, jj2] == qmv)
        qm[:, m] = np.where(vism & ~dup, qmv, 196)
    Fg = M1ext[qm[:, 0]] + M1ext[qm[:, 1]] + M1ext[qm[:, 2]]
    mn_g = Fg.min(axis=1)
    mx_g = Fg.max(axis=1)
    d_g = (mx_g - mn_g).astype(np.float64)
    r_g = mn_qext[qm].sum(axis=1) - mn_g
    ttg = (Fg - mn_g[:, None]) / d_g[:, None]
    hostD += float((ttg.astype(np.float64) ** 2).sum())

    # ---- per-core staging buffers ----
    JR = np.zeros((NCORES, JCAP, 196), np.float32)
    MJ = np.full((NCORES, JCAP), SENT, np.float32)
    GR = np.zeros((NCORES, GCAP, 196), np.float32)
    MG = np.zeros((NCORES, GCAP, 8), np.float32)
    MG[:, :, 0:3] = SENT

    core_j = bs // BS
    core_g = gb // BS
    for i in range(NCORES):
        selj = core_j == i
        nj = int(selj.sum())
        assert nj <= JCAP, f"joint rows {nj} > capacity {JCAP}"
        JR[i, :nj] = h[bs[selj], js[selj]]
        MJ[i, :nj] = qj[selj].astype(np.float32)

        selg = core_g == i
        ng = int(selg.sum())
        assert ng <= GCAP, f"group rows {ng} > capacity {GCAP}"
        GR[i, :ng] = h[gb[selg], 14 + gg[selg]]
        qmi = qm[selg]
        MG[i, :ng, 0:3] = np.where(qmi == 196, SENT, qmi).astype(np.float32)
        MG[i, :ng, 3] = (1.0 / d_g[selg]).astype(np.float32)
        MG[i, :ng, 4] = (r_g[selg] / d_g[selg]).astype(np.float32)

    # pack iota + per-tile metadata into one [128, 516] constant block:
    # [:, 0:196] iota, [:, 196+t] joint q of row 128t+p, [:, 260+8t+c] group meta
    CONST = np.zeros((NCORES, 128, 516), np.float32)
    CONST[:, :, 0:196] = np.arange(196, dtype=np.float32)[None, None, :]
    CONST[:, :, 196:196 + JTILES] = MJ.reshape(NCORES, JTILES, 128).transpose(0, 2, 1)
    CONST[:, :, 260:516] = (
        MG.reshape(NCORES, GTILES, 128, 8).transpose(0, 2, 1, 3).reshape(NCORES, 128, GTILES * 8)
    )
    n1 = float(v.sum()) / 2.0
    return JR, GR, CONST, hostD, n1


# ---------------------------------------------------------------- device prog
_nc_cache = None

# CONST block layout (free-dim offsets in the [128, 516] constant tile)
C_IOTA = 0        # [0:196]   iota 0..195
C_MJ = 196        # [196+t]   joint q for row 128t+p
C_MG = 260        # [260+8t+c] group meta (q0,q1,q2,wq,c0,_,_,_)
C_NC = 516

HS_SLOTS = 4  # h super-tile double buffering depth


def _schedule():
    """Global super order: interleave 2 joint : 1 group."""
    order = []
    ji, gi = 0, 0
    while ji < JSUP or gi < GSUP:
        for _ in range(2):
            if ji < JSUP:
                order.append(("J", ji))
                ji += 1
        if gi < GSUP:
            order.append(("G", gi))
            gi += 1
    return order


def _build_nc():
    global _nc_cache
    if _nc_cache is not None:
        return _nc_cache

    nc = bass.Bass()
    JRd = nc.declare_dram_parameter("JR", [JCAP, 196], F32, isOutput=False)
    GRd = nc.declare_dram_parameter("GR", [GCAP, 196], F32, isOutput=False)
    CONSTd = nc.declare_dram_parameter("CONST", [128, C_NC], F32, isOutput=False)
    SJd = nc.declare_dram_parameter("SJ", [196, 196], F32, isOutput=True)
    SGd = nc.declare_dram_parameter("SG", [196, 197], F32, isOutput=True)
    SQd = nc.declare_dram_parameter("SQ", [128, JSUP + GSUP], F32, isOutput=True)

    eq = mybir.AluOpType.is_equal
    mul = mybir.AluOpType.mult
    order = _schedule()
    NSUP = len(order)
    NW = JTILES + 3 * GTILES  # total W tiles (160)

    # per-(kind, super) bookkeeping shared by all engine programs:
    # number of W builds completed once this tile's builds are done, and the
    # W-slot index of each build.  Build order == PE consumption order.
    wslot = {}  # (kind, tile_idx, m) -> W slot
    wthresh = {}  # (kind, tile_idx) -> builds done incl. this tile
    nb = 0
    for kind, T in order:
        for s in range(4):
            if kind == "J":
                t = 4 * T + s
                wslot[("J", t, 0)] = nb
                nb += 1
                wthresh[("J", t)] = nb
            else:
                t = 4 * T + s
                for m in range(3):
                    wslot[("G", t, m)] = nb
                    nb += 1
                wthresh[("G", t)] = nb
    assert nb == NW

    with (
        nc.sbuf_tensor("cst", [128, C_NC], F32) as cst,
        nc.sbuf_tensor("hsb", [128, HS_SLOTS, 4, 196], F32) as hsb,
        nc.sbuf_tensor("wall", [128, NW, 196], F32) as wall,
        nc.sbuf_tensor("sq_sb", [128, JSUP + GSUP], F32) as sq_sb,
        nc.sbuf_tensor("sj_sb", [98, 2, 196], F32) as sj_sb,
        nc.sbuf_tensor("sg_sb", [98, 2, 197], F32) as sg_sb,
        nc.psum_tensor("psjl", [98, 196], F32) as psjl,
        nc.psum_tensor("psjh", [98, 196], F32) as psjh,
        nc.psum_tensor("psgl", [98, 197], F32) as psgl,
        nc.psum_tensor("psgh", [98, 197], F32) as psgh,
        nc.psum_tensor("pscr", [128, 784], F32) as pscr,
        nc.semaphore("s_cst") as s_cst,
        nc.semaphore("s_h0") as s_h0,
        nc.semaphore("s_h1") as s_h1,
        nc.semaphore("s_h2") as s_h2,
        nc.semaphore("s_h3") as s_h3,
        nc.semaphore("s_w") as s_w,
        nc.semaphore("s_pe") as s_pe,
        nc.semaphore("s_act") as s_act,
        nc.semaphore("s_out") as s_out,
        nc.Block() as block,
    ):
        s_h = [s_h0, s_h1, s_h2, s_h3]

        def hs_dram(kind, T):
            d = JRd if kind == "J" else GRd
            return d[512 * T:512 * (T + 1), :].rearrange("(s p) c -> p s c", p=128)

        @block.sync
        def _(sync):
            sync.dma_start(out=cst[:], in_=CONSTd[:]).then_inc(s_cst, 16)
            for i, (kind, T) in enumerate(order):
                slot = i % HS_SLOTS
                if i >= HS_SLOTS:
                    # slot-reuse guard: consumers of use (i - HS_SLOTS) done
                    sync.wait_ge(s_pe, i - HS_SLOTS + 1)
                    sync.wait_ge(s_act, i - HS_SLOTS + 1)
                sync.dma_start(out=hsb[:, slot], in_=hs_dram(kind, T)).then_inc(
                    s_h[slot], 16
                )
            # outputs
            sync.wait_ge(s_w, NW + 4)
            sync.dma_start(
                out=SJd[:].rearrange("(c p) q -> p c q", p=98), in_=sj_sb[:]
            ).then_inc(s_out, 16)
            sync.dma_start(
                out=SGd[:].rearrange("(c p) q -> p c q", p=98), in_=sg_sb[:]
            ).then_inc(s_out, 16)
            sync.wait_ge(s_act, NSUP)
            sync.dma_start(out=SQd[:], in_=sq_sb[:]).then_inc(s_out, 16)
            sync.wait_ge(s_out, 48)

        @block.vector
        def _(vector):
            vector.wait_ge(s_cst, 16)
            for kind, T in order:
                for s in range(4):
                    t = 4 * T + s
                    if kind == "J":
                        w = wall[:, wslot[("J", t, 0)], :]
                        vector.tensor_scalar(
                            out=w, in0=cst[:, C_IOTA:C_IOTA + 196],
                            scalar1=cst[:, C_MJ + t:C_MJ + t + 1], scalar2=None,
                            op0=eq,
                        ).then_inc(s_w, 1)
                    else:
                        mo = C_MG + 8 * t
                        for m in range(3):
                            w = wall[:, wslot[("G", t, m)], :]
                            vector.tensor_scalar(
                                out=w, in0=cst[:, C_IOTA:C_IOTA + 196],
                                scalar1=cst[:, mo + m:mo + m + 1],
                                scalar2=cst[:, mo + 3:mo + 4],
                                op0=eq, op1=mul,
                            ).then_inc(s_w, 1)
            # final PSUM -> SBUF copies
            vector.wait_ge(s_pe, NSUP)
            vector.tensor_copy(sj_sb[:, 0, :], psjl[:]).then_inc(s_w, 1)
            vector.tensor_copy(sj_sb[:, 1, :], psjh[:]).then_inc(s_w, 1)
            vector.tensor_copy(sg_sb[:, 0, :], psgl[:]).then_inc(s_w, 1)
            vector.tensor_copy(sg_sb[:, 1, :], psgh[:]).then_inc(s_w, 1)

        @block.scalar
        def _(scalar):
            for i, (kind, T) in enumerate(order):
                slot = i % HS_SLOTS
                scalar.wait_ge(s_h[slot], 16 * (i // HS_SLOTS + 1))
                col = T if kind == "J" else JSUP + T
                scalar.activation(
                    pscr[:], hsb[:, slot].rearrange("p a b -> p (a b)"),
                    mybir.ActivationFunctionType.Square,
                    accum_out=sq_sb[:, col:col + 1],
                ).then_inc(s_act, 1)

        @block.tensor
        def _(tensor):
            cnt = {}
            tot = {"jl": JTILES, "jh": JTILES,
                   "gl": GTILES * 3, "gh": GTILES * 3,
                   "gl_c0": GTILES, "gh_c0": GTILES}

            def mm(region, out_ap, lhsT, rhs, inc=None):
                c = cnt.get(region, 0)
                cnt[region] = c + 1
                r = nc.tensor.matmul(
                    out=out_ap, lhsT=lhsT, rhs=rhs,
                    start=(c == 0), stop=(c == tot[region] - 1),
                    skip_group_check=True,
                )
                if inc is not None:
                    r.then_inc(inc, 1)
                return r

            for i, (kind, T) in enumerate(order):
                slot = i % HS_SLOTS
                tensor.wait_ge(s_h[slot], 16 * (i // HS_SLOTS + 1))
                hs = hsb[:, slot]
                for s in range(4):
                    t = 4 * T + s
                    tensor.wait_ge(s_w, wthresh[(kind, t)])
                    last = s == 3
                    if kind == "J":
                        w = wall[:, wslot[("J", t, 0)], :]
                        mm("jl", psjl[:, 0:196], hs[:, s, 0:98], w)
                        mm("jh", psjh[:, 0:196], hs[:, s, 98:196], w,
                           inc=s_pe if last else None)
                    else:
                        mo = C_MG + 8 * t
                        for m in range(3):
                            w = wall[:, wslot[("G", t, m)], :]
                            mm("gl", psgl[:, 0:196], hs[:, s, 0:98], w)
                            mm("gh", psgh[:, 0:196], hs[:, s, 98:196], w)
                        mm("gl_c0", psgl[:, 196:197], hs[:, s, 0:98],
                           cst[:, mo + 4:mo + 5])
                        mm("gh_c0", psgh[:, 196:197], hs[:, s, 98:196],
                           cst[:, mo + 4:mo + 5], inc=s_pe if last else None)

    _nc_cache = nc
    return nc


# ---------------------------------------------------------------- entry point
LAST = {}


def kernel(os, h, t, v):
    h = np.asarray(h)
    t = np.asarray(t)
    v = np.asarray(v)
    JR, GR, CONST, hostD, n1 = _host_stage(h, t, v)
    nc = _build_nc()
    in_maps = [
        {"JR": JR[i], "GR": GR[i], "CONST": CONST[i]}
        for i in range(NCORES)
    ]
    res = run_bass_kernel_spmd(
        nc, in_maps, list(range(NCORES)),
        trace=bool(_os.environ.get("KERNEL_TRACE")),
    )
    LAST["res"] = res

    M1p, mn_q, d_q = _tables()[0:3]
    MJT = (M1p / d_q[:, None]).T  # [196 p, 196 q] joint table (weight folded)
    MGT = M1p.T                   # [196 p, 196 q] group table
    total = 0.0
    for i in range(NCORES):
        out = res.results[i]
        SJ = out["SJ"].astype(np.float64)
        SG = out["SG"].astype(np.float64)
        termA = float(out["SQ"].astype(np.float64).sum())
        termB = float((SJ * MJT).sum() + (SG[:, :196] * MGT).sum()
                      + SG[:, 196].sum())
        total += termA - 2.0 * termB
    total += hostD
    return np.float32(total / n1)



# revision 25
# speedup vs baseline: 3.5908x; 3.5908x over previous
"""Trainium2 kernel for nn_MeanSquaredError2 (scatter_memory) — sorted-scatter.

Math: the reference builds, per (batch, channel), a gaussian-filtered one-hot
target map tt, min-max normalizes it, masks by visibility, and returns
sum(mask*(h-tt)^2) / (v.sum()/2).

Factorization:
  sum mask*(h-tt)^2 = sum_vis h^2 - 2*sum_vis <h, tt> + sum_vis tt^2
The filtered one-hot of pixel q is a fixed 196-value table row M1[q]; with
M1' = M1 - min(M1[q]) each channel's <h, tt> decomposes into per-pixel terms
w*<h, M1'[q]> (w = 1/denom) plus, for group channels, (r/d)*rowsum(h) which —
like sum_vis tt^2 and v.sum() — is cheap O(rows) host work.

Device work (the h-heavy part): one "entry" per (visible channel, pixel):
  SU[q, p] += w_e * h_e[p]        (scatter over pixel q)
  SQ       += sum_p h_e[p]^2      (unique entries only)
Entries are sorted by q host-side so each 128-entry tile scatters into a
narrow q-window: one matmul per tile with the tiny one-hot weight matrix
stationary (LDWEIGHTS of ~8-16 columns) and the h tile streaming (196 cols).
PSUM rows are the q axis (psA: q<128, psB: q>=128), accumulated across all
tiles via has_written start=False accumulation. h rows ship as fp8e4 (the
final scalar tolerates ~1e-3), W as bf16. Sum-of-squares runs concurrently
on Scalar/Vector/GpSimd engines over the unique-entry prefix of the stream.
Host finishes with the [196,196] table contraction sum(SU * M1').
"""

import os as _os
import sys
import numpy as np

for _p in ("/opt/trn_rl_repo", "/root/.axon_site/_ro/trn_rl_repo"):
    if _p not in sys.path:
        sys.path.append(_p)

import ml_dtypes  # noqa: E402
import concourse.bass as bass  # noqa: E402
from concourse import mybir  # noqa: E402
from concourse.bass_utils import run_bass_kernel_spmd  # noqa: E402

COL = 14
NJ = 14
RADIUS = 4
B = 8192
NCORES = 8
BS = B // NCORES

F32 = mybir.dt.float32
BF16 = mybir.dt.bfloat16
DT_H = mybir.dt.float8e4          # entry-row dtype on device
NP_H = ml_dtypes.float8_e4m3
NP_BF = ml_dtypes.bfloat16

CH = 8            # tiles per DMA chunk
NSLOT = 4         # chunk double-buffer depth
NWARM = 16        # PE warm-up matmuls during the DMA prologue


# ---------------------------------------------------------------- host tables
_tables_cache = None


def _tables():
    global _tables_cache
    if _tables_cache is not None:
        return _tables_cache
    x = np.arange(-RADIUS, RADIUS + 1).astype(np.float32)
    k = np.exp(-0.5 * x * x)
    k = (k / k.sum()).astype(np.float32)
    Km = np.zeros((COL, COL), np.float32)
    for p in range(COL):
        v = np.zeros(COL, np.float32)
        v[p] = 1.0
        vp = np.pad(v, RADIUS, mode="symmetric")
        Km[:, p] = np.convolve(vp, k[::-1], mode="valid").astype(np.float32)
    M1 = np.zeros((196, 196), np.float32)
    for yi in range(COL):
        for xi in range(COL):
            M1[yi * COL + xi] = np.outer(Km[:, yi], Km[:, xi]).reshape(196)
    mn_q = M1.min(axis=1)
    d_q = M1.max(axis=1) - mn_q
    M1p = (M1 - mn_q[:, None]).astype(np.float64)
    T2j = ((M1p / d_q[:, None]) ** 2).sum(axis=1)
    M1ext = np.concatenate([M1, np.zeros((1, 196), np.float32)])
    mn_qext = np.concatenate([mn_q, [0.0]]).astype(np.float64)
    _tables_cache = (M1p, mn_q, d_q, T2j, M1ext, mn_qext)
    return _tables_cache


def _host_stage(h, t, v):
    """Build per-core sorted entry streams + W tiles, plus host scalar terms.

    Returns (ER, WT, meta, hostD, termB_host, n1) where meta carries the
    compile-time program shape shared by all cores.
    """
    M1p, mn_q, d_q, T2j, M1ext, mn_qext = _tables()
    h = np.ascontiguousarray(h, dtype=np.float32).reshape(B, 18, 196)
    ti = t.astype(np.float32) * COL
    idx = np.clip(ti.astype(np.int32), 0, COL - 1)
    xi, yi = idx[..., 0], idx[..., 1]
    vis = v[..., 0] == 1  # [B, NJ]
    q = yi.astype(np.int64) * COL + xi.astype(np.int64)  # [B, NJ]

    # ---- joints ----
    bj = np.argwhere(vis)
    bs_, js_ = bj[:, 0], bj[:, 1]
    qj = q[bs_, js_]
    hostD = float(T2j[qj].sum())

    # ---- groups (deduped pixel sets, 196 = absent) ----
    gvis = vis[:, :12].reshape(B, 4, 3).any(axis=2)
    bg = np.argwhere(gvis)
    gb, gg = bg[:, 0], bg[:, 1]
    n_g = len(gb)
    qm = np.full((n_g, 3), 196, np.int64)
    for m in range(3):
        jj = gg * 3 + m
        vism = vis[gb, jj]
        qmv = q[gb, jj]
        dup = np.zeros(n_g, bool)
        for m2 in range(m):
            jj2 = gg * 3 + m2
            dup |= vis[gb, jj2] & (q[gb, jj2] == qmv)
        qm[:, m] = np.where(vism & ~dup, qmv, 196)
    Fg = M1ext[qm[:, 0]] + M1ext[qm[:, 1]] + M1ext[qm[:, 2]]
    mn_g = Fg.min(axis=1)
    mx_g = Fg.max(axis=1)
    d_g = (mx_g - mn_g).astype(np.float64)
    r_g = mn_qext[qm].sum(axis=1) - mn_g
    ttg = (Fg - mn_g[:, None]) / d_g[:, None]
    hostD += float((ttg.astype(np.float64) ** 2).sum())

    # group rowsum term, on the original f32 h (host-side O(rows) work)
    grows = h[gb, 14 + gg].astype(np.float64)
    termB_host = float(((r_g / d_g) * grows.sum(axis=1)).sum())

    # ---- per-core entry lists ----
    per_core = []  # (uq, urows_idx, uw, dq, drows_idx, dw) with sorted q
    for c in range(NCORES):
        lo, hi = c * BS, (c + 1) * BS
        selj = (bs_ >= lo) & (bs_ < hi)
        selg = (gb >= lo) & (gb < hi)
        qmc = qm[selg]
        gidx = np.stack([gb[selg], 14 + gg[selg]], axis=1)  # h row index [n,2]
        wgc = 1.0 / d_g[selg]
        valid = qmc != 196
        first_m = valid.argmax(axis=1)            # first valid pixel (exists)
        rows_u = [np.stack([bs_[selj], js_[selj]], axis=1)]
        q_u = [qj[selj]]
        w_u = [1.0 / d_q[qj[selj]]]
        rows_u.append(gidx)
        q_u.append(qmc[np.arange(len(qmc)), first_m])
        w_u.append(wgc)
        rows_d, q_d, w_d = [], [], []
        for m in range(3):
            extra = valid[:, m] & (first_m != m)
            rows_d.append(gidx[extra])
            q_d.append(qmc[extra, m])
            w_d.append(wgc[extra])
        uq = np.concatenate(q_u)
        ur = np.concatenate(rows_u)
        uw = np.concatenate(w_u)
        dq_ = np.concatenate(q_d)
        dr = np.concatenate(rows_d)
        dw = np.concatenate(w_d)
        o = np.argsort(uq, kind="stable")
        od = np.argsort(dq_, kind="stable")
        per_core.append((uq[o], ur[o], uw[o], dq_[od], dr[od], dw[od]))

    # ---- 32-wide q-blocks; tiles never cross a block (PE out base must be
    # 32-aligned and <96 within its psum tensor) ----
    BLK = [(0, 32), (32, 32), (64, 32), (96, 32), (128, 32), (160, 32), (192, 4)]
    NB = len(BLK)

    def blk_counts(seg_q):
        return [int(((seg_q >= b0) & (seg_q < b0 + bw)).sum()) for b0, bw in BLK]

    ntb = np.zeros((2, NB), np.int64)  # [seg, block] union tile counts
    for uq, _, _, dq_, _, _ in per_core:
        for s, seg_q in ((0, uq), (1, dq_)):
            for b, n in enumerate(blk_counts(seg_q)):
                ntb[s, b] = max(ntb[s, b], -(-n // 128))
    NTU = int(ntb[0].sum())
    NT = NTU + int(ntb[1].sum())
    NCH = -(-NT // CH)
    SPW = 32

    # per-tile block index (global tile order: unique blocks, then dup blocks)
    tile_blk = []
    for s in range(2):
        for b in range(NB):
            tile_blk += [b] * int(ntb[s, b])
    tile_blk = np.array(tile_blk, np.int64)
    # first tile index of (seg, block)
    t0 = np.zeros((2, NB), np.int64)
    acc = 0
    for s in range(2):
        for b in range(NB):
            t0[s, b] = acc
            acc += int(ntb[s, b])

    # ---- staging buffers ----
    ER = np.zeros((NCORES, 128, NT, 196), NP_H)
    WT = np.zeros((NCORES, 128, NT, SPW), NP_BF)
    for c, (uq, ur, uw, dq_, dr, dw) in enumerate(per_core):
        for s, (seg_q, seg_r, seg_w) in ((0, (uq, ur, uw)), (1, (dq_, dr, dw))):
            for b, (b0, bw) in enumerate(BLK):
                sel = (seg_q >= b0) & (seg_q < b0 + bw)
                n = int(sel.sum())
                if n == 0:
                    continue
                bq, br, bwt = seg_q[sel], seg_r[sel], seg_w[sel]
                tt_ = t0[s, b] + np.arange(n) // 128
                pp = np.arange(n) % 128
                ER[c, pp, tt_, :] = h[br[:, 0], br[:, 1]].astype(NP_H)
                WT[c, pp, tt_, bq - b0] = bwt.astype(NP_BF)

    # ---- SQ engine assignment over unique chunks ----
    # chunk c covers tiles [8c, min(8c+8, NT)); unique prefix = ∩ [0, NTU)
    sq_chunks = []  # (chunk, k_unique)
    for c in range(NCH):
        k = min(NTU - c * CH, CH, NT - c * CH)
        if k > 0:
            sq_chunks.append((c, k))
    # round-robin weighted (GpSimd can't do fused square+accum on this
    # compiler build; ACT is slightly faster per element than DVE)
    pattern = ["act", "dve", "act", "dve", "act", "act", "dve"]
    sq_assign = {c: pattern[i % len(pattern)] for i, (c, k) in enumerate(sq_chunks)}
    sq_k = {c: k for c, k in sq_chunks}

    meta = dict(
        NT=NT, NTU=NTU, NCH=NCH, SPW=SPW,
        tile_blk=tuple(int(x) for x in tile_blk),
        sq_assign=tuple(sorted(sq_assign.items())),
        sq_k=tuple(sorted(sq_k.items())),
    )
    n1 = float(v.sum()) / 2.0
    return ER, WT, meta, hostD, termB_host, n1


# ---------------------------------------------------------------- device prog
_nc_cache = {}


def _build_nc(meta):
    key = (meta["NT"], meta["NTU"], meta["SPW"], meta["tile_blk"],
           meta["sq_assign"], meta["sq_k"])
    if key in _nc_cache:
        return _nc_cache[key]

    NT, NTU, NCH, SPW = meta["NT"], meta["NTU"], meta["NCH"], meta["SPW"]
    tile_blk = meta["tile_blk"]
    sq_assign = dict(meta["sq_assign"])
    sq_k = dict(meta["sq_k"])

    nc = bass.Bass()
    ERd = nc.declare_dram_parameter("ER", [128, NT, 196], DT_H, isOutput=False)
    WTd = nc.declare_dram_parameter("WT", [128, NT, SPW], BF16, isOutput=False)
    SUAd = nc.declare_dram_parameter("SUA", [96, 196], F32, isOutput=True)
    SUBd = nc.declare_dram_parameter("SUB", [96, 196], F32, isOutput=True)
    SUCd = nc.declare_dram_parameter("SUC", [4, 196], F32, isOutput=True)
    SQd = nc.declare_dram_parameter("SQ", [128, NCH], F32, isOutput=True)

    ntiles = lambda c: min(CH, NT - c * CH)  # noqa: E731

    # block -> (psum tensor id, base partition within it, width)
    def blk_dst(b):
        if b < 3:
            return 0, b * 32, 32
        if b < 6:
            return 1, (b - 3) * 32, 32
        return 2, 0, 4

    # last matmul per psum tensor (for stop flags)
    last = [-1, -1, -1]
    for t in range(NT):
        last[blk_dst(tile_blk[t])[0]] = t

    # per-engine cumulative SQ-op count after each chunk (slot-reuse waits)
    engs = ("act", "dve", "gp")
    sq_done_after = {e: [] for e in engs}
    acc = {e: 0 for e in engs}
    for c in range(NCH):
        e = sq_assign.get(c)
        if e is not None:
            acc[e] += 1
        for e2 in engs:
            sq_done_after[e2].append(acc[e2])
    total_sq = {e: acc[e] for e in engs}

    from contextlib import ExitStack

    with ExitStack() as ctx:
        ee = ctx.enter_context
        ersb = ee(nc.sbuf_tensor("ersb", [128, NSLOT, CH, 196], DT_H))
        wall = ee(nc.sbuf_tensor("wall", [128, NT, SPW], BF16))
        zt = ee(nc.sbuf_tensor("zt", [128, 196], DT_H))
        scr_act = ee(nc.sbuf_tensor("scr_act", [128, CH, 196], BF16))
        scr_dve = ee(nc.sbuf_tensor("scr_dve", [128, CH, 196], BF16))
        scr_gp = ee(nc.sbuf_tensor("scr_gp", [128, CH, 196], BF16))
        sq_sb = ee(nc.sbuf_tensor("sq_sb", [128, NCH], F32))
        sua_sb = ee(nc.sbuf_tensor("sua_sb", [96, 196], F32))
        sub_sb = ee(nc.sbuf_tensor("sub_sb", [96, 196], F32))
        suc_sb = ee(nc.sbuf_tensor("suc_sb", [4, 196], F32))
        psA = ee(nc.psum_tensor("psA", [96, 196], F32))
        psB = ee(nc.psum_tensor("psB", [96, 196], F32))
        psC = ee(nc.psum_tensor("psC", [4, 196], F32))
        ps = [psA, psB, psC]
        s_wt = ee(nc.semaphore("s_wt"))
        s_e = [ee(nc.semaphore(f"s_e{i}")) for i in range(NSLOT)]
        s_z = ee(nc.semaphore("s_z"))
        s_pe = ee(nc.semaphore("s_pe"))
        s_eng = {e: ee(nc.semaphore(f"s_{e}")) for e in engs}
        s_cp = ee(nc.semaphore("s_cp"))
        s_out = ee(nc.semaphore("s_out"))
        block = ee(nc.Block())

        @block.sync
        def _(sync):
            sync.dma_start(out=wall[:], in_=WTd[:]).then_inc(s_wt, 16)
            for c in range(NCH):
                slot = c % NSLOT
                if c >= NSLOT:
                    pc = c - NSLOT
                    sync.wait_ge(s_pe, pc + 1)
                    for e in engs:
                        if sq_done_after[e][pc] > 0:
                            sync.wait_ge(s_eng[e], sq_done_after[e][pc])
                k = ntiles(c)
                sync.dma_start(
                    out=ersb[:, slot, 0:k, :], in_=ERd[:, c * CH:c * CH + k, :]
                ).then_inc(s_e[slot], 16)
            # outputs
            sync.wait_ge(s_cp, 3)
            sync.dma_start(out=SUAd[:], in_=sua_sb[:]).then_inc(s_out, 16)
            sync.dma_start(out=SUBd[:], in_=sub_sb[:]).then_inc(s_out, 16)
            sync.dma_start(out=SUCd[:], in_=suc_sb[:]).then_inc(s_out, 16)
            for e in engs:
                if total_sq[e] > 0:
                    sync.wait_ge(s_eng[e], total_sq[e])
            sync.dma_start(out=SQd[:], in_=sq_sb[:]).then_inc(s_out, 16)
            sync.wait_ge(s_out, 64)

        @block.tensor
        def _(tensor):
            tensor.wait_ge(s_z, 1)
            # warm-up + PSUM init: zeros into psA/psB/psC
            for i in range(NWARM):
                j = i % 3
                tensor.matmul(
                    out=ps[j][:, 0:196], lhsT=zt[:, 0:(96 if j < 2 else 4)],
                    rhs=zt[:, 0:196],
                    start=(i < 3), stop=False, skip_group_check=True,
                )
            tensor.wait_ge(s_wt, 16)
            for c in range(NCH):
                slot = c % NSLOT
                tensor.wait_ge(s_e[slot], 16 * (c // NSLOT + 1))
                k = ntiles(c)
                r = None
                for i in range(k):
                    t = c * CH + i
                    pj, base, w = blk_dst(tile_blk[t])
                    r = tensor.matmul(
                        out=ps[pj][base:base + w, :], lhsT=wall[:, t, 0:w],
                        rhs=ersb[:, slot, i, :],
                        start=False, stop=(t == last[pj]),
                        skip_group_check=True,
                    )
                r.then_inc(s_pe, 1)

        @block.scalar
        def _(scalar):
            for c in range(NCH):
                if sq_assign.get(c) != "act":
                    continue
                slot = c % NSLOT
                scalar.wait_ge(s_e[slot], 16 * (c // NSLOT + 1))
                k = sq_k[c]
                scalar.activation(
                    scr_act[:, 0:k, :], ersb[:, slot, 0:k, :],
                    mybir.ActivationFunctionType.Square,
                    accum_out=sq_sb[:, c:c + 1],
                ).then_inc(s_eng["act"], 1)

        @block.vector
        def _(vector):
            vector.memset(zt[:], 0.0).then_inc(s_z, 1)
            for c in range(NCH):
                if sq_assign.get(c) != "dve":
                    continue
                slot = c % NSLOT
                vector.wait_ge(s_e[slot], 16 * (c // NSLOT + 1))
                k = sq_k[c]
                vector.scalar_tensor_tensor(
                    out=scr_dve[:, 0:k, :],
                    in0=ersb[:, slot, 0:k, :], scalar=0.0,
                    in1=ersb[:, slot, 0:k, :],
                    op0=mybir.AluOpType.bypass, op1=mybir.AluOpType.mult,
                    accum_out=sq_sb[:, c:c + 1],
                ).then_inc(s_eng["dve"], 1)
            vector.wait_ge(s_pe, NCH)
            vector.tensor_copy(sua_sb[:], psA[:]).then_inc(s_cp, 1)
            vector.tensor_copy(sub_sb[:], psB[:]).then_inc(s_cp, 1)
            vector.tensor_copy(suc_sb[:], psC[:]).then_inc(s_cp, 1)

    _nc_cache[key] = nc
    return nc


# ---------------------------------------------------------------- entry point
LAST = {}


def kernel(os, h, t, v):
    h = np.asarray(h)
    t = np.asarray(t)
    v = np.asarray(v)
    ER, WT, meta, hostD, termB_host, n1 = _host_stage(h, t, v)
    nc = _build_nc(meta)
    in_maps = [{"ER": ER[i], "WT": WT[i]} for i in range(NCORES)]
    res = run_bass_kernel_spmd(
        nc, in_maps, list(range(NCORES)),
        trace=bool(_os.environ.get("KERNEL_TRACE")),
    )
    LAST["res"] = res

    M1p = _tables()[0]
    sq_cols = [c for c, _ in meta["sq_k"]]
    termA = 0.0
    termB_scatter = 0.0
    for i in range(NCORES):
        out = res.results[i]
        termA += float(out["SQ"].astype(np.float64)[:, sq_cols].sum())
        SU = np.concatenate([out["SUA"], out["SUB"], out["SUC"]], axis=0)
        termB_scatter += float((SU.astype(np.float64) * M1p).sum())
    total = termA - 2.0 * (termB_scatter + termB_host) + hostD
    return np.float32(total / n1)


# revision 42
# speedup vs baseline: 3.9260x; 1.0933x over previous
"""Trainium2 kernel for nn_MeanSquaredError2 (scatter_memory) — sorted-scatter.

Math: the reference builds, per (batch, channel), a gaussian-filtered one-hot
target map tt, min-max normalizes it, masks by visibility, and returns
sum(mask*(h-tt)^2) / (v.sum()/2).

Factorization:
  sum mask*(h-tt)^2 = sum_vis h^2 - 2*sum_vis <h, tt> + sum_vis tt^2
The filtered one-hot of pixel q is a fixed 196-value table row M1[q]; with
M1' = M1 - min(M1[q]) each channel's <h, tt> decomposes into per-pixel terms
w*<h, M1'[q]> (w = 1/denom) plus, for group channels, (r/d)*rowsum(h) which —
like sum_vis tt^2 and v.sum() — is cheap O(rows) host work.

Device work (the h-heavy part): one "entry" per (visible channel, pixel):
  SU[q, p] += w_e * h_e[p]        (scatter over pixel q)
  SQ       += sum_p h_e[p]^2      (unique entries only)
Entries are sorted by q host-side so each 128-entry tile scatters into a
narrow q-window: one matmul per tile with the tiny one-hot weight matrix
stationary (LDWEIGHTS of ~8-16 columns) and the h tile streaming (196 cols).
PSUM rows are the q axis (psA: q<128, psB: q>=128), accumulated across all
tiles via has_written start=False accumulation. h rows ship as fp8e4 (the
final scalar tolerates ~1e-3), W as bf16. Sum-of-squares runs concurrently
on Scalar/Vector/GpSimd engines over the unique-entry prefix of the stream.
Host finishes with the [196,196] table contraction sum(SU * M1').
"""

import os as _os
import sys
import numpy as np

for _p in ("/opt/trn_rl_repo", "/root/.axon_site/_ro/trn_rl_repo"):
    if _p not in sys.path:
        sys.path.append(_p)

import ml_dtypes  # noqa: E402
import concourse.bass as bass  # noqa: E402
from concourse import mybir  # noqa: E402
from concourse.bass_utils import run_bass_kernel_spmd  # noqa: E402

COL = 14
NJ = 14
RADIUS = 4
B = 8192
NCORES = 8
BS = B // NCORES

F32 = mybir.dt.float32
BF16 = mybir.dt.bfloat16
DT_H = mybir.dt.float8e4          # entry-row dtype on device
NP_H = ml_dtypes.float8_e4m3
NP_BF = ml_dtypes.bfloat16

CH = 8            # tiles per DMA chunk
NSLOT = 4         # chunk double-buffer depth
NWARM = 12        # PE warm-up matmuls during the DMA prologue


# ---------------------------------------------------------------- host tables
_tables_cache = None


def _tables():
    global _tables_cache
    if _tables_cache is not None:
        return _tables_cache
    x = np.arange(-RADIUS, RADIUS + 1).astype(np.float32)
    k = np.exp(-0.5 * x * x)
    k = (k / k.sum()).astype(np.float32)
    Km = np.zeros((COL, COL), np.float32)
    for p in range(COL):
        v = np.zeros(COL, np.float32)
        v[p] = 1.0
        vp = np.pad(v, RADIUS, mode="symmetric")
        Km[:, p] = np.convolve(vp, k[::-1], mode="valid").astype(np.float32)
    M1 = np.zeros((196, 196), np.float32)
    for yi in range(COL):
        for xi in range(COL):
            M1[yi * COL + xi] = np.outer(Km[:, yi], Km[:, xi]).reshape(196)
    mn_q = M1.min(axis=1)
    d_q = M1.max(axis=1) - mn_q
    M1p = (M1 - mn_q[:, None]).astype(np.float64)
    T2j = ((M1p / d_q[:, None]) ** 2).sum(axis=1)
    M1ext = np.concatenate([M1, np.zeros((1, 196), np.float32)])
    mn_qext = np.concatenate([mn_q, [0.0]]).astype(np.float64)
    _tables_cache = (M1p, mn_q, d_q, T2j, M1ext, mn_qext)
    return _tables_cache


def _host_stage(h, t, v):
    """Build per-core sorted entry streams + W tiles, plus host scalar terms.

    Returns (ER, WT, meta, hostD, termB_host, n1) where meta carries the
    compile-time program shape shared by all cores.
    """
    M1p, mn_q, d_q, T2j, M1ext, mn_qext = _tables()
    h = np.ascontiguousarray(h, dtype=np.float32).reshape(B, 18, 196)
    ti = t.astype(np.float32) * COL
    idx = np.clip(ti.astype(np.int32), 0, COL - 1)
    xi, yi = idx[..., 0], idx[..., 1]
    vis = v[..., 0] == 1  # [B, NJ]
    q = yi.astype(np.int64) * COL + xi.astype(np.int64)  # [B, NJ]

    # ---- joints ----
    bj = np.argwhere(vis)
    bs_, js_ = bj[:, 0], bj[:, 1]
    qj = q[bs_, js_]
    hostD = float(T2j[qj].sum())

    # ---- groups (deduped pixel sets, 196 = absent) ----
    gvis = vis[:, :12].reshape(B, 4, 3).any(axis=2)
    bg = np.argwhere(gvis)
    gb, gg = bg[:, 0], bg[:, 1]
    n_g = len(gb)
    qm = np.full((n_g, 3), 196, np.int64)
    for m in range(3):
        jj = gg * 3 + m
        vism = vis[gb, jj]
        qmv = q[gb, jj]
        dup = np.zeros(n_g, bool)
        for m2 in range(m):
            jj2 = gg * 3 + m2
            dup |= vis[gb, jj2] & (q[gb, jj2] == qmv)
        qm[:, m] = np.where(vism & ~dup, qmv, 196)
    Fg = M1ext[qm[:, 0]] + M1ext[qm[:, 1]] + M1ext[qm[:, 2]]
    mn_g = Fg.min(axis=1)
    mx_g = Fg.max(axis=1)
    d_g = (mx_g - mn_g).astype(np.float64)
    r_g = mn_qext[qm].sum(axis=1) - mn_g
    ttg = (Fg - mn_g[:, None]) / d_g[:, None]
    hostD += float((ttg.astype(np.float64) ** 2).sum())

    # group rowsum term, on the original f32 h (host-side O(rows) work)
    grows = h[gb, 14 + gg].astype(np.float64)
    termB_host = float(((r_g / d_g) * grows.sum(axis=1)).sum())

    # ---- per-core entry lists ----
    per_core = []  # (uq, urows_idx, uw, dq, drows_idx, dw) with sorted q
    for c in range(NCORES):
        lo, hi = c * BS, (c + 1) * BS
        selj = (bs_ >= lo) & (bs_ < hi)
        selg = (gb >= lo) & (gb < hi)
        qmc = qm[selg]
        gidx = np.stack([gb[selg], 14 + gg[selg]], axis=1)  # h row index [n,2]
        wgc = 1.0 / d_g[selg]
        valid = qmc != 196
        first_m = valid.argmax(axis=1)            # first valid pixel (exists)
        rows_u = [np.stack([bs_[selj], js_[selj]], axis=1)]
        q_u = [qj[selj]]
        w_u = [1.0 / d_q[qj[selj]]]
        rows_u.append(gidx)
        q_u.append(qmc[np.arange(len(qmc)), first_m])
        w_u.append(wgc)
        rows_d, q_d, w_d = [], [], []
        for m in range(3):
            extra = valid[:, m] & (first_m != m)
            rows_d.append(gidx[extra])
            q_d.append(qmc[extra, m])
            w_d.append(wgc[extra])
        uq = np.concatenate(q_u)
        ur = np.concatenate(rows_u)
        uw = np.concatenate(w_u)
        dq_ = np.concatenate(q_d)
        dr = np.concatenate(rows_d)
        dw = np.concatenate(w_d)
        o = np.argsort(uq, kind="stable")
        od = np.argsort(dq_, kind="stable")
        per_core.append((uq[o], ur[o], uw[o], dq_[od], dr[od], dw[od]))

    # ---- 32-wide q-blocks; tiles never cross a block (PE out base must be
    # 32-aligned and <96 within its psum tensor) ----
    BLK = [(0, 32), (32, 32), (64, 32), (96, 32), (128, 32), (160, 32), (192, 4)]
    NB = len(BLK)

    def blk_counts(seg_q):
        return [int(((seg_q >= b0) & (seg_q < b0 + bw)).sum()) for b0, bw in BLK]

    ntb = np.zeros((2, NB), np.int64)  # [seg, block] union tile counts
    for uq, _, _, dq_, _, _ in per_core:
        for s, seg_q in ((0, uq), (1, dq_)):
            for b, n in enumerate(blk_counts(seg_q)):
                ntb[s, b] = max(ntb[s, b], -(-n // 128))
    NTU = int(ntb[0].sum())
    NT = NTU + int(ntb[1].sum())
    NCH = -(-NT // CH)
    SPW = 32

    # per-tile block index (global tile order: unique blocks, then dup blocks)
    tile_blk = []
    for s in range(2):
        for b in range(NB):
            tile_blk += [b] * int(ntb[s, b])
    tile_blk = np.array(tile_blk, np.int64)
    # first tile index of (seg, block)
    t0 = np.zeros((2, NB), np.int64)
    acc = 0
    for s in range(2):
        for b in range(NB):
            t0[s, b] = acc
            acc += int(ntb[s, b])

    # ---- union W width per tile (block-relative; out base must stay
    # 32-aligned so W carries leading zeros from the block base) ----
    wid = np.ones(NT, np.int64)
    for uq, _, _, dq_, _, _ in per_core:
        for s, seg_q in ((0, uq), (1, dq_)):
            for b, (b0, bw) in enumerate(BLK):
                sel = (seg_q >= b0) & (seg_q < b0 + bw)
                n = int(sel.sum())
                if n == 0:
                    continue
                bq = seg_q[sel]
                tt_ = np.arange(n) // 128
                for ti in range(int(tt_[-1]) + 1):
                    g = t0[s, b] + ti
                    wid[g] = max(wid[g], int(bq[tt_ == ti].max()) - b0 + 1)
    # round widths to multiples of 4 (8-byte-aligned LDWEIGHTS bases; the
    # extra columns hold zero weights and write zero into in-block rows)
    blk_w = np.array([bw for _, bw in BLK], np.int64)
    wid = np.minimum(-(-wid // 4) * 4, blk_w[tile_blk])
    wof = np.zeros(NT, np.int64)
    acc = 0
    for g in range(NT):
        wof[g] = acc
        acc += int(wid[g])
    WTOT = acc

    # ---- staging buffers ----
    ER = np.zeros((NCORES, 128, NT, 196), NP_H)
    WT = np.zeros((NCORES, 128, WTOT), NP_H)
    for c, (uq, ur, uw, dq_, dr, dw) in enumerate(per_core):
        for s, (seg_q, seg_r, seg_w) in ((0, (uq, ur, uw)), (1, (dq_, dr, dw))):
            for b, (b0, bw) in enumerate(BLK):
                sel = (seg_q >= b0) & (seg_q < b0 + bw)
                n = int(sel.sum())
                if n == 0:
                    continue
                bq, br, bwt = seg_q[sel], seg_r[sel], seg_w[sel]
                tt_ = t0[s, b] + np.arange(n) // 128
                pp = np.arange(n) % 128
                ER[c, pp, tt_, :] = h[br[:, 0], br[:, 1]].astype(NP_H)
                WT[c, pp, wof[tt_] + bq - b0] = bwt.astype(NP_H)

    # ---- SQ engine assignment over unique chunks ----
    # chunk c covers tiles [8c, min(8c+8, NT)); unique prefix = ∩ [0, NTU)
    sq_chunks = []  # (chunk, k_unique)
    for c in range(NCH):
        k = min(NTU - c * CH, CH, NT - c * CH)
        if k > 0:
            sq_chunks.append((c, k))
    # weighted round-robin: ACT is fastest per element; GpSimd squares into
    # a scratch (no fused accum on Pool) and DVE reduces it
    pattern = ["act", "dve", "act", "gp", "act", "dve", "gp", "act", "act",
               "gp", "act", "dve"]
    sq_assign = {c: pattern[i % len(pattern)] for i, (c, k) in enumerate(sq_chunks)}
    sq_k = {c: k for c, k in sq_chunks}

    # DMA piece sizes in chunks: small first (fast pipeline start), larger
    # later (fewer descriptor-generation slices on the sync ring)
    ps_sizes = []
    rem = NCH
    for s_ in (1, 1, 2, 2):
        if rem <= 0:
            break
        s_ = min(s_, rem)
        ps_sizes.append(s_)
        rem -= s_
    while rem > 0:
        s_ = min(3, rem)
        ps_sizes.append(s_)
        rem -= s_

    meta = dict(
        NT=NT, NTU=NTU, NCH=NCH, WTOT=WTOT,
        tile_blk=tuple(int(x) for x in tile_blk),
        wid=tuple(int(x) for x in wid),
        wof=tuple(int(x) for x in wof),
        ps_sizes=tuple(ps_sizes),
        sq_assign=tuple(sorted(sq_assign.items())),
        sq_k=tuple(sorted(sq_k.items())),
    )
    n1 = float(v.sum()) / 2.0
    return ER, WT, meta, hostD, termB_host, n1


# ---------------------------------------------------------------- device prog
_nc_cache = {}


def _build_nc(meta):
    key = (meta["NT"], meta["NTU"], meta["tile_blk"], meta["wid"],
           meta["ps_sizes"], meta["sq_assign"], meta["sq_k"])
    if key in _nc_cache:
        return _nc_cache[key]

    NT, NTU, NCH, WTOT = meta["NT"], meta["NTU"], meta["NCH"], meta["WTOT"]
    tile_blk, wid, wof = meta["tile_blk"], meta["wid"], meta["wof"]
    ps_sizes = meta["ps_sizes"]
    sq_assign = dict(meta["sq_assign"])
    sq_k = dict(meta["sq_k"])
    NP = len(ps_sizes)

    nc = bass.Bass()
    ERd = nc.declare_dram_parameter("ER", [128, NT, 196], DT_H, isOutput=False)
    WTd = nc.declare_dram_parameter("WT", [128, WTOT], DT_H, isOutput=False)
    SUAd = nc.declare_dram_parameter("SUA", [96, 196], F32, isOutput=True)
    SUBd = nc.declare_dram_parameter("SUB", [96, 196], F32, isOutput=True)
    SUCd = nc.declare_dram_parameter("SUC", [4, 196], F32, isOutput=True)
    SQd = nc.declare_dram_parameter("SQ", [128, NCH], F32, isOutput=True)

    # block -> (psum tensor id, base partition within it)
    def blk_dst(b):
        if b < 3:
            return 0, b * 32
        if b < 6:
            return 1, (b - 3) * 32
        return 2, 0

    # last matmul per psum tensor (for stop flags + early output copies)
    last = [-1, -1, -1]
    for t in range(NT):
        last[blk_dst(tile_blk[t])[0]] = t
    assert all(x >= 0 for x in last)

    # chunk/tile -> piece mapping
    piece_of_chunk = []
    for p, s_ in enumerate(ps_sizes):
        piece_of_chunk += [p] * s_
    piece_first_tile = [0] * NP
    acc = 0
    for p, s_ in enumerate(ps_sizes):
        piece_first_tile[p] = acc
        acc += s_ * CH
    piece_last_tile = [min(piece_first_tile[p] + ps_sizes[p] * CH, NT) - 1
                       for p in range(NP)]

    # Input stream: WT, tinyW, ER0, tiny0, ER1, tiny1, ER2..ERn, fence.
    # Every item increments s_er by 16. Item k's data is trusted only once
    # item k+1's semaphore fires: per-(ring,engine) FIFO means the
    # successor's sem descriptors provably follow item k's data
    # descriptors, and small successors' completion receipts return much
    # sooner than a large item's own receipt. (Gating on an item's own sem
    # raced on hardware.)
    items = [("wt",), ("tinyw",)]
    for p in range(NP):
        items.append(("er", p))
        if p < 2:
            items.append(("tiny", p))
    items.append(("fence",))
    item_idx = {it: i for i, it in enumerate(items)}
    th_wt = 16 * (item_idx[("wt",)] + 2)
    th_piece = [16 * (item_idx[("er", p)] + 2) for p in range(NP)]

    engs = ("act", "dve", "gp")
    total_sq = {e: sum(1 for c in sq_assign if sq_assign[c] == e) for e in engs}
    gp_chunks = sorted(c for c in sq_assign if sq_assign[c] == "gp")
    # SQ column owners for the final DMA wait: act ops + dve ops (incl.
    # the reduces of gp chunks, which DVE performs)
    dve_ops_total = total_sq["dve"] + total_sq["gp"]

    from contextlib import ExitStack

    with ExitStack() as ctx:
        ee = ctx.enter_context
        ersb = ee(nc.sbuf_tensor("ersb", [128, NT, 196], DT_H))
        wall = ee(nc.sbuf_tensor("wall", [128, WTOT], DT_H))
        zt = ee(nc.sbuf_tensor("zt", [128, 196], DT_H))
        fence_sb = ee(nc.sbuf_tensor("fence_sb", [128, 196], DT_H))
        scr_act = ee(nc.sbuf_tensor("scr_act", [128, CH, 196], BF16))
        scr_dve = ee(nc.sbuf_tensor("scr_dve", [128, CH, 196], BF16))
        scr_gp = ee(nc.sbuf_tensor("scr_gp", [128, len(gp_chunks), CH, 196],
                                   BF16)) if gp_chunks else None
        sq_sb = ee(nc.sbuf_tensor("sq_sb", [128, NCH], F32))
        sua_sb = ee(nc.sbuf_tensor("sua_sb", [96, 196], F32))
        sub_sb = ee(nc.sbuf_tensor("sub_sb", [96, 196], F32))
        suc_sb = ee(nc.sbuf_tensor("suc_sb", [4, 196], F32))
        psA = ee(nc.psum_tensor("psA", [96, 196], F32))
        psB = ee(nc.psum_tensor("psB", [96, 196], F32))
        psC = ee(nc.psum_tensor("psC", [4, 196], F32))
        ps = [psA, psB, psC]
        su_sb = [sua_sb, sub_sb, suc_sb]
        s_er = ee(nc.semaphore("s_er"))
        s_z = ee(nc.semaphore("s_z"))
        s_pes = [ee(nc.semaphore(f"s_pe{j}")) for j in range(3)]
        s_eng = {e: ee(nc.semaphore(f"s_{e}")) for e in engs}
        s_gpd = ee(nc.semaphore("s_gpd"))
        s_cp = ee(nc.semaphore("s_cp"))
        s_out = ee(nc.semaphore("s_out"))
        block = ee(nc.Block())

        @block.sync
        def _(sync):
            for it in items:
                if it[0] == "wt":
                    d = sync.dma_start(out=wall[:], in_=WTd[:])
                elif it[0] == "tinyw":
                    d = sync.dma_start(out=fence_sb[:], in_=WTd[:, 0:196])
                elif it[0] == "er":
                    p = it[1]
                    t_lo = piece_first_tile[p]
                    t_hi = piece_last_tile[p] + 1
                    d = sync.dma_start(out=ersb[:, t_lo:t_hi, :],
                                       in_=ERd[:, t_lo:t_hi, :])
                elif it[0] == "tiny":
                    d = sync.dma_start(out=fence_sb[:],
                                       in_=ERd[:, piece_last_tile[it[1]], :])
                else:  # fence
                    d = sync.dma_start(out=fence_sb[:], in_=ERd[:, 0, :])
                d.then_inc(s_er, 16)
            # outputs: SQ as soon as all SQ ops done, SU per-tensor as the
            # copies land (copies are emitted in A, B, C order)
            sync.wait_ge(s_eng["act"], total_sq["act"])
            sync.wait_ge(s_eng["dve"], dve_ops_total)
            sync.dma_start(out=SQd[:], in_=sq_sb[:]).then_inc(s_out, 16)
            for j, dst in enumerate((SUAd, SUBd, SUCd)):
                sync.wait_ge(s_cp, j + 1)
                sync.dma_start(out=dst[:], in_=su_sb[j][:]).then_inc(s_out, 16)
            sync.wait_ge(s_out, 64)

        @block.tensor
        def _(tensor):
            tensor.wait_ge(s_z, 1)
            # warm-up + PSUM init: zeros into psA/psB/psC
            for i in range(NWARM):
                j = i % 3
                tensor.matmul(
                    out=ps[j][:, 0:196], lhsT=zt[:, 0:(96 if j < 2 else 4)],
                    rhs=zt[:, 0:196],
                    start=(i < 3), stop=False, skip_group_check=True,
                )
            tensor.wait_ge(s_er, th_wt)
            prev_piece = -1
            for t in range(NT):
                piece = piece_of_chunk[t // CH]
                if piece != prev_piece:
                    tensor.wait_ge(s_er, th_piece[piece])
                    prev_piece = piece
                pj, base = blk_dst(tile_blk[t])
                w = wid[t]
                r = tensor.matmul(
                    out=ps[pj][base:base + w, :],
                    lhsT=wall[:, wof[t]:wof[t] + w],
                    rhs=ersb[:, t, :],
                    start=False, stop=(t == last[pj]),
                    skip_group_check=True,
                )
                if t in last:
                    r.then_inc(s_pes[last.index(t)], 1)

        @block.scalar
        def _(scalar):
            # dummy op: pull the Square LUT into the table cache during the
            # DMA prologue instead of on the critical path
            scalar.activation(scr_act[:, 0, 0:1], zt[:, 0:1],
                              mybir.ActivationFunctionType.Square)
            for c in range(NCH):
                if sq_assign.get(c) != "act":
                    continue
                scalar.wait_ge(s_er, th_piece[piece_of_chunk[c]])
                k = sq_k[c]
                scalar.activation(
                    scr_act[:, 0:k, :], ersb[:, c * CH:c * CH + k, :],
                    mybir.ActivationFunctionType.Square,
                    accum_out=sq_sb[:, c:c + 1],
                ).then_inc(s_eng["act"], 1)

        @block.gpsimd
        def _(gpsimd):
            for gi, c in enumerate(gp_chunks):
                gpsimd.wait_ge(s_er, th_piece[piece_of_chunk[c]])
                k = sq_k[c]
                gpsimd.tensor_tensor(
                    out=scr_gp[:, gi, 0:k, :],
                    in0=ersb[:, c * CH:c * CH + k, :],
                    in1=ersb[:, c * CH:c * CH + k, :],
                    op=mybir.AluOpType.mult,
                ).then_inc(s_eng["gp"], 1)

        @block.vector
        def _(vector):
            vector.memset(zt[:], 0.0).then_inc(s_z, 1)
            ngp = 0
            for c in range(NCH):
                e = sq_assign.get(c)
                if e == "dve":
                    vector.wait_ge(s_er, th_piece[piece_of_chunk[c]])
                    k = sq_k[c]
                    vector.scalar_tensor_tensor(
                        out=scr_dve[:, 0:k, :],
                        in0=ersb[:, c * CH:c * CH + k, :], scalar=0.0,
                        in1=ersb[:, c * CH:c * CH + k, :],
                        op0=mybir.AluOpType.bypass, op1=mybir.AluOpType.mult,
                        accum_out=sq_sb[:, c:c + 1],
                    ).then_inc(s_eng["dve"], 1)
                elif e == "gp":
                    ngp += 1
                    vector.wait_ge(s_eng["gp"], ngp)
                    k = sq_k[c]
                    vector.tensor_reduce(
                        out=sq_sb[:, c:c + 1],
                        in_=scr_gp[:, ngp - 1, 0:k, :],
                        axis=mybir.AxisListType.XY, op=mybir.AluOpType.add,
                    ).then_inc(s_eng["dve"], 1)
            for j in range(3):
                vector.wait_ge(s_pes[j], 1)
                vector.tensor_copy(su_sb[j][:], ps[j][:]).then_inc(s_cp, 1)

    _nc_cache[key] = nc
    return nc


# ---------------------------------------------------------------- entry point
LAST = {}


def kernel(os, h, t, v):
    h = np.asarray(h)
    t = np.asarray(t)
    v = np.asarray(v)
    ER, WT, meta, hostD, termB_host, n1 = _host_stage(h, t, v)
    nc = _build_nc(meta)
    in_maps = [{"ER": ER[i], "WT": WT[i]} for i in range(NCORES)]
    res = run_bass_kernel_spmd(
        nc, in_maps, list(range(NCORES)),
        trace=bool(_os.environ.get("KERNEL_TRACE")),
    )
    LAST["res"] = res

    M1p = _tables()[0]
    sq_cols = [c for c, _ in meta["sq_k"]]
    termA = 0.0
    termB_scatter = 0.0
    for i in range(NCORES):
        out = res.results[i]
        termA += float(out["SQ"].astype(np.float64)[:, sq_cols].sum())
        SU = np.concatenate([out["SUA"], out["SUB"], out["SUC"]], axis=0)
        termB_scatter += float((SU.astype(np.float64) * M1p).sum())
    total = termA - 2.0 * (termB_scatter + termB_host) + hostD
    return np.float32(total / n1)


# revision 53
# speedup vs baseline: 4.4897x; 1.1436x over previous
"""Trainium2 kernel for nn_MeanSquaredError2 (scatter_memory) — sorted-scatter.

Math: the reference builds, per (batch, channel), a gaussian-filtered one-hot
target map tt, min-max normalizes it, masks by visibility, and returns
sum(mask*(h-tt)^2) / (v.sum()/2).

Factorization:
  sum mask*(h-tt)^2 = sum_vis h^2 - 2*sum_vis <h, tt> + sum_vis tt^2
The filtered one-hot of pixel q is a fixed 196-value table row M1[q]; with
M1' = M1 - min(M1[q]) each channel's <h, tt> decomposes into per-pixel terms
w*<h, M1'[q]> (w = 1/denom) plus, for group channels, (r/d)*rowsum(h) which —
like sum_vis tt^2 and v.sum() — is cheap O(rows) host work.

Device work (the h-heavy part): one "entry" per (visible channel, pixel):
  SU[q, p] += w_e * h_e[p]        (scatter over pixel q)
  SQ       += sum_p h_e[p]^2      (unique entries only)
Entries are sorted by q host-side so each 128-entry tile scatters into a
narrow q-window: one matmul per tile with the tiny one-hot weight matrix
stationary (LDWEIGHTS of ~8-16 columns) and the h tile streaming (196 cols).
PSUM rows are the q axis (psA: q<128, psB: q>=128), accumulated across all
tiles via has_written start=False accumulation. h rows ship as fp8e4 (the
final scalar tolerates ~1e-3), W as bf16. Sum-of-squares runs concurrently
on Scalar/Vector/GpSimd engines over the unique-entry prefix of the stream.
Host finishes with the [196,196] table contraction sum(SU * M1').
"""

import os as _os
import sys
import numpy as np

for _p in ("/opt/trn_rl_repo", "/root/.axon_site/_ro/trn_rl_repo"):
    if _p not in sys.path:
        sys.path.append(_p)

import ml_dtypes  # noqa: E402
import concourse.bass as bass  # noqa: E402
from concourse import mybir  # noqa: E402
from concourse.bass_utils import run_bass_kernel_spmd  # noqa: E402

COL = 14
NJ = 14
RADIUS = 4
B = 8192
NCORES = 8
BS = B // NCORES

F32 = mybir.dt.float32
BF16 = mybir.dt.bfloat16
DT_H = mybir.dt.float8e4          # entry-row dtype on device
NP_H = ml_dtypes.float8_e4m3
NP_BF = ml_dtypes.bfloat16

CH = 8            # tiles per DMA chunk
NSLOT = 4         # chunk double-buffer depth
NWARM = 36        # PE warm-up matmuls during the DMA prologue (also holds
                  # the DVFS activity window open until real work arrives)


# ---------------------------------------------------------------- host tables
_tables_cache = None


def _tables():
    global _tables_cache
    if _tables_cache is not None:
        return _tables_cache
    x = np.arange(-RADIUS, RADIUS + 1).astype(np.float32)
    k = np.exp(-0.5 * x * x)
    k = (k / k.sum()).astype(np.float32)
    Km = np.zeros((COL, COL), np.float32)
    for p in range(COL):
        v = np.zeros(COL, np.float32)
        v[p] = 1.0
        vp = np.pad(v, RADIUS, mode="symmetric")
        Km[:, p] = np.convolve(vp, k[::-1], mode="valid").astype(np.float32)
    M1 = np.zeros((196, 196), np.float32)
    for yi in range(COL):
        for xi in range(COL):
            M1[yi * COL + xi] = np.outer(Km[:, yi], Km[:, xi]).reshape(196)
    mn_q = M1.min(axis=1)
    d_q = M1.max(axis=1) - mn_q
    M1p = (M1 - mn_q[:, None]).astype(np.float64)
    T2j = ((M1p / d_q[:, None]) ** 2).sum(axis=1)
    M1ext = np.concatenate([M1, np.zeros((1, 196), np.float32)])
    mn_qext = np.concatenate([mn_q, [0.0]]).astype(np.float64)
    _tables_cache = (M1p, mn_q, d_q, T2j, M1ext, mn_qext)
    return _tables_cache


def _host_stage(h, t, v):
    """Build per-core sorted entry streams + W tiles, plus host scalar terms.

    Returns (ER, WT, meta, hostD, termB_host, n1) where meta carries the
    compile-time program shape shared by all cores.
    """
    M1p, mn_q, d_q, T2j, M1ext, mn_qext = _tables()
    h = np.ascontiguousarray(h, dtype=np.float32).reshape(B, 18, 196)
    ti = t.astype(np.float32) * COL
    idx = np.clip(ti.astype(np.int32), 0, COL - 1)
    xi, yi = idx[..., 0], idx[..., 1]
    vis = v[..., 0] == 1  # [B, NJ]
    q = yi.astype(np.int64) * COL + xi.astype(np.int64)  # [B, NJ]

    # ---- joints ----
    bj = np.argwhere(vis)
    bs_, js_ = bj[:, 0], bj[:, 1]
    qj = q[bs_, js_]
    hostD = float(T2j[qj].sum())

    # ---- groups (deduped pixel sets, 196 = absent) ----
    gvis = vis[:, :12].reshape(B, 4, 3).any(axis=2)
    bg = np.argwhere(gvis)
    gb, gg = bg[:, 0], bg[:, 1]
    n_g = len(gb)
    qm = np.full((n_g, 3), 196, np.int64)
    for m in range(3):
        jj = gg * 3 + m
        vism = vis[gb, jj]
        qmv = q[gb, jj]
        dup = np.zeros(n_g, bool)
        for m2 in range(m):
            jj2 = gg * 3 + m2
            dup |= vis[gb, jj2] & (q[gb, jj2] == qmv)
        qm[:, m] = np.where(vism & ~dup, qmv, 196)
    Fg = M1ext[qm[:, 0]] + M1ext[qm[:, 1]] + M1ext[qm[:, 2]]
    mn_g = Fg.min(axis=1)
    mx_g = Fg.max(axis=1)
    d_g = (mx_g - mn_g).astype(np.float64)
    r_g = mn_qext[qm].sum(axis=1) - mn_g
    ttg = (Fg - mn_g[:, None]) / d_g[:, None]
    hostD += float((ttg.astype(np.float64) ** 2).sum())

    # group rowsum term, on the original f32 h (host-side O(rows) work)
    grows = h[gb, 14 + gg].astype(np.float64)
    termB_host = float(((r_g / d_g) * grows.sum(axis=1)).sum())

    # ---- per-core entry lists ----
    per_core = []  # (uq, urows_idx, uw, dq, drows_idx, dw) with sorted q
    for c in range(NCORES):
        lo, hi = c * BS, (c + 1) * BS
        selj = (bs_ >= lo) & (bs_ < hi)
        selg = (gb >= lo) & (gb < hi)
        qmc = qm[selg]
        gidx = np.stack([gb[selg], 14 + gg[selg]], axis=1)  # h row index [n,2]
        wgc = 1.0 / d_g[selg]
        valid = qmc != 196
        first_m = valid.argmax(axis=1)            # first valid pixel (exists)
        rows_u = [np.stack([bs_[selj], js_[selj]], axis=1)]
        q_u = [qj[selj]]
        w_u = [1.0 / d_q[qj[selj]]]
        rows_u.append(gidx)
        q_u.append(qmc[np.arange(len(qmc)), first_m])
        w_u.append(wgc)
        rows_d, q_d, w_d = [], [], []
        for m in range(3):
            extra = valid[:, m] & (first_m != m)
            rows_d.append(gidx[extra])
            q_d.append(qmc[extra, m])
            w_d.append(wgc[extra])
        uq = np.concatenate(q_u)
        ur = np.concatenate(rows_u)
        uw = np.concatenate(w_u)
        dq_ = np.concatenate(q_d)
        dr = np.concatenate(rows_d)
        dw = np.concatenate(w_d)
        o = np.argsort(uq, kind="stable")
        od = np.argsort(dq_, kind="stable")
        per_core.append((uq[o], ur[o], uw[o], dq_[od], dr[od], dw[od]))

    # ---- 32-wide q-blocks; tiles never cross a block (PE out base must be
    # 32-aligned and <96 within its psum tensor) ----
    BLK = [(0, 32), (32, 32), (64, 32), (96, 32), (128, 32), (160, 32), (192, 4)]
    NB = len(BLK)

    def blk_counts(seg_q):
        return [int(((seg_q >= b0) & (seg_q < b0 + bw)).sum()) for b0, bw in BLK]

    ntb = np.zeros((2, NB), np.int64)  # [seg, block] union tile counts
    for uq, _, _, dq_, _, _ in per_core:
        for s, seg_q in ((0, uq), (1, dq_)):
            for b, n in enumerate(blk_counts(seg_q)):
                ntb[s, b] = max(ntb[s, b], -(-n // 128))
    NTU = int(ntb[0].sum())
    NT = NTU + int(ntb[1].sum())
    NCH = -(-NT // CH)
    SPW = 32

    # per-tile block index (global tile order: unique blocks, then dup blocks)
    tile_blk = []
    for s in range(2):
        for b in range(NB):
            tile_blk += [b] * int(ntb[s, b])
    tile_blk = np.array(tile_blk, np.int64)
    # first tile index of (seg, block)
    t0 = np.zeros((2, NB), np.int64)
    acc = 0
    for s in range(2):
        for b in range(NB):
            t0[s, b] = acc
            acc += int(ntb[s, b])

    # ---- union W width per tile (block-relative; out base must stay
    # 32-aligned so W carries leading zeros from the block base) ----
    wid = np.ones(NT, np.int64)
    for uq, _, _, dq_, _, _ in per_core:
        for s, seg_q in ((0, uq), (1, dq_)):
            for b, (b0, bw) in enumerate(BLK):
                sel = (seg_q >= b0) & (seg_q < b0 + bw)
                n = int(sel.sum())
                if n == 0:
                    continue
                bq = seg_q[sel]
                tt_ = np.arange(n) // 128
                for ti in range(int(tt_[-1]) + 1):
                    g = t0[s, b] + ti
                    wid[g] = max(wid[g], int(bq[tt_ == ti].max()) - b0 + 1)
    # round widths to multiples of 4 (8-byte-aligned LDWEIGHTS bases; the
    # extra columns hold zero weights and write zero into in-block rows)
    blk_w = np.array([bw for _, bw in BLK], np.int64)
    wid = np.minimum(-(-wid // 4) * 4, blk_w[tile_blk])
    wof = np.zeros(NT, np.int64)
    acc = 0
    for g in range(NT):
        wof[g] = acc
        acc += int(wid[g])
    WTOT = acc

    # ---- staging buffers ----
    ER = np.zeros((NCORES, 128, NT, 196), NP_H)
    WT = np.zeros((NCORES, 128, WTOT), NP_H)
    for c, (uq, ur, uw, dq_, dr, dw) in enumerate(per_core):
        for s, (seg_q, seg_r, seg_w) in ((0, (uq, ur, uw)), (1, (dq_, dr, dw))):
            for b, (b0, bw) in enumerate(BLK):
                sel = (seg_q >= b0) & (seg_q < b0 + bw)
                n = int(sel.sum())
                if n == 0:
                    continue
                bq, br, bwt = seg_q[sel], seg_r[sel], seg_w[sel]
                tt_ = t0[s, b] + np.arange(n) // 128
                pp = np.arange(n) % 128
                ER[c, pp, tt_, :] = h[br[:, 0], br[:, 1]].astype(NP_H)
                WT[c, pp, wof[tt_] + bq - b0] = bwt.astype(NP_H)

    # ---- SQ engine assignment over unique chunks ----
    # chunk c covers tiles [8c, min(8c+8, NT)); unique prefix = ∩ [0, NTU)
    sq_chunks = []  # (chunk, k_unique)
    for c in range(NCH):
        k = min(NTU - c * CH, CH, NT - c * CH)
        if k > 0:
            sq_chunks.append((c, k))
    # weighted ACT/DVE round-robin. No GpSimd: Pool shares SBUF read ports
    # with DVE, so Pool streaming ops just steal DVE bandwidth (measured).
    pattern = ["act", "dve", "act", "act", "dve", "act", "dve", "act", "act",
               "dve", "act", "dve"]
    sq_assign = {c: pattern[i % len(pattern)] for i, (c, k) in enumerate(sq_chunks)}
    sq_k = {c: k for c, k in sq_chunks}

    # DMA piece sizes in chunks: small first (fast pipeline start), larger
    # later (fewer descriptor-generation slices on the sync ring)
    ps_sizes = []
    rem = NCH
    for s_ in (1, 1, 2, 2):
        if rem <= 0:
            break
        s_ = min(s_, rem)
        ps_sizes.append(s_)
        rem -= s_
    while rem > 0:
        s_ = min(3, rem)
        ps_sizes.append(s_)
        rem -= s_

    meta = dict(
        NT=NT, NTU=NTU, NCH=NCH, WTOT=WTOT,
        tile_blk=tuple(int(x) for x in tile_blk),
        wid=tuple(int(x) for x in wid),
        wof=tuple(int(x) for x in wof),
        ps_sizes=tuple(ps_sizes),
        sq_assign=tuple(sorted(sq_assign.items())),
        sq_k=tuple(sorted(sq_k.items())),
    )
    n1 = float(v.sum()) / 2.0
    return ER, WT, meta, hostD, termB_host, n1


# ---------------------------------------------------------------- device prog
_nc_cache = {}


def _build_nc(meta):
    key = (meta["NT"], meta["NTU"], meta["tile_blk"], meta["wid"],
           meta["ps_sizes"], meta["sq_assign"], meta["sq_k"])
    if key in _nc_cache:
        return _nc_cache[key]

    NT, NTU, NCH, WTOT = meta["NT"], meta["NTU"], meta["NCH"], meta["WTOT"]
    tile_blk, wid, wof = meta["tile_blk"], meta["wid"], meta["wof"]
    ps_sizes = meta["ps_sizes"]
    sq_assign = dict(meta["sq_assign"])
    sq_k = dict(meta["sq_k"])
    NP = len(ps_sizes)

    nc = bass.Bass()
    ERd = nc.declare_dram_parameter("ER", [128, NT, 196], DT_H, isOutput=False)
    WTd = nc.declare_dram_parameter("WT", [128, WTOT], DT_H, isOutput=False)
    # single packed output: [sua 0:196 | sub 196:392 | suc 392:588 | sq ...]
    OCOL = 588 + NCH
    OUTd = nc.declare_dram_parameter("OUT", [128, OCOL], F32, isOutput=True)

    # block -> (psum tensor id, base partition within it)
    def blk_dst(b):
        if b < 3:
            return 0, b * 32
        if b < 6:
            return 1, (b - 3) * 32
        return 2, 0

    # last matmul per psum tensor (for stop flags + early output copies)
    last = [-1, -1, -1]
    for t in range(NT):
        last[blk_dst(tile_blk[t])[0]] = t
    assert all(x >= 0 for x in last)

    # chunk/tile -> piece mapping
    piece_of_chunk = []
    for p, s_ in enumerate(ps_sizes):
        piece_of_chunk += [p] * s_
    piece_first_tile = [0] * NP
    acc = 0
    for p, s_ in enumerate(ps_sizes):
        piece_first_tile[p] = acc
        acc += s_ * CH
    piece_last_tile = [min(piece_first_tile[p] + ps_sizes[p] * CH, NT) - 1
                       for p in range(NP)]

    # Input stream: WT, tinyW, then each ER piece followed by a tiny
    # successor. Every item increments s_er by 16. Item k's data is trusted
    # only once item k+1's semaphore fires: per-(ring,engine) FIFO means the
    # successor's sem descriptors provably follow item k's data descriptors,
    # and a tiny successor (one descriptor per SBUF port group) completes
    # its receipt far sooner than a large item's own receipt. (Gating on an
    # item's own sem raced on hardware.)
    items = [("wt",), ("tiny", -1)]
    for p in range(NP):
        items.append(("er", p))
        if p < NP - 2:
            items.append(("tiny", p))
    items.append(("tiny", NP - 1))  # trailing fence for the last piece
    item_idx = {it: i for i, it in enumerate(items)}
    th_wt = 16 * (item_idx[("wt",)] + 2)
    th_piece = [16 * (item_idx[("er", p)] + 2) for p in range(NP)]

    engs = ("act", "dve")
    total_sq = {e: sum(1 for c in sq_assign if sq_assign[c] == e) for e in engs}

    from contextlib import ExitStack

    with ExitStack() as ctx:
        ee = ctx.enter_context
        ersb = ee(nc.sbuf_tensor("ersb", [128, NT, 196], DT_H))
        wall = ee(nc.sbuf_tensor("wall", [128, WTOT], DT_H))
        zt = ee(nc.sbuf_tensor("zt", [128, 196], DT_H))
        fence_sb = ee(nc.sbuf_tensor("fence_sb", [128, 196], DT_H))
        scr_act = ee(nc.sbuf_tensor("scr_act", [128, CH, 196], BF16))
        scr_dve = ee(nc.sbuf_tensor("scr_dve", [128, CH, 196], BF16))
        out_sb = ee(nc.sbuf_tensor("out_sb", [128, OCOL], F32))
        psA = ee(nc.psum_tensor("psA", [96, 196], F32))
        psB = ee(nc.psum_tensor("psB", [96, 196], F32))
        psC = ee(nc.psum_tensor("psC", [4, 196], F32))
        ps = [psA, psB, psC]
        s_er = ee(nc.semaphore("s_er"))
        s_z = ee(nc.semaphore("s_z"))
        s_pes = [ee(nc.semaphore(f"s_pe{j}")) for j in range(3)]
        s_eng = {e: ee(nc.semaphore(f"s_{e}")) for e in engs}
        s_cp = ee(nc.semaphore("s_cp"))
        s_out = ee(nc.semaphore("s_out"))
        block = ee(nc.Block())

        # stride-8 partition views: one element per SBUF port group, so a
        # tiny DMA puts exactly one descriptor on each of the 16 SDMA
        # engines — its semaphore then proves every engine drained the
        # preceding item's descriptors (FIFO per engine ring).

        @block.sync
        def _(sync):
            for it in items:
                if it[0] == "wt":
                    d = sync.dma_start(out=wall[:], in_=WTd[:])
                elif it[0] == "er":
                    p = it[1]
                    t_lo = piece_first_tile[p]
                    t_hi = piece_last_tile[p] + 1
                    d = sync.dma_start(out=ersb[:, t_lo:t_hi, :],
                                       in_=ERd[:, t_lo:t_hi, :])
                else:  # tiny successor (full 128 partitions: every SDMA
                    # engine must carry one of its descriptors for the
                    # FIFO-ordering guarantee — 16-desc variants raced)
                    src_t = piece_last_tile[max(it[1], 0)]
                    d = sync.dma_start(out=fence_sb[:], in_=ERd[:, src_t, :])
                d.then_inc(s_er, 16)
            # single packed output once every producer is done
            sync.wait_ge(s_eng["act"], total_sq["act"])
            sync.wait_ge(s_eng["dve"], total_sq["dve"])
            sync.wait_ge(s_cp, 3)
            sync.dma_start(out=OUTd[:], in_=out_sb[:]).then_inc(s_out, 16)
            sync.wait_ge(s_out, 16)

        @block.tensor
        def _(tensor):
            tensor.wait_ge(s_z, 1)
            # warm-up + PSUM init: zeros into psA/psB/psC. Narrow streams so
            # many cheap matmuls bridge the DMA prologue and hold the DVFS
            # activity window open.
            for i in range(NWARM):
                j = i % 3
                tensor.matmul(
                    out=ps[j][:, 0:196] if i < 3 else ps[j][:, 0:64],
                    lhsT=zt[:, 0:(96 if j < 2 else 4)],
                    rhs=zt[:, 0:196] if i < 3 else zt[:, 0:64],
                    start=(i < 3), stop=False, skip_group_check=True,
                )
            tensor.wait_ge(s_er, th_wt)
            prev_piece = -1
            for t in range(NT):
                piece = piece_of_chunk[t // CH]
                if piece != prev_piece:
                    tensor.wait_ge(s_er, th_piece[piece])
                    prev_piece = piece
                pj, base = blk_dst(tile_blk[t])
                w = wid[t]
                r = tensor.matmul(
                    out=ps[pj][base:base + w, :],
                    lhsT=wall[:, wof[t]:wof[t] + w],
                    rhs=ersb[:, t, :],
                    start=False, stop=(t == last[pj]),
                    skip_group_check=True,
                )
                if t in last:
                    r.then_inc(s_pes[last.index(t)], 1)

        @block.scalar
        def _(scalar):
            # dummy op: pull the Square LUT into the table cache during the
            # DMA prologue instead of on the critical path
            scalar.activation(scr_act[:, 0, 0:1], zt[:, 0:1],
                              mybir.ActivationFunctionType.Square)
            for c in range(NCH):
                if sq_assign.get(c) != "act":
                    continue
                scalar.wait_ge(s_er, th_piece[piece_of_chunk[c]])
                k = sq_k[c]
                scalar.activation(
                    scr_act[:, 0:k, :], ersb[:, c * CH:c * CH + k, :],
                    mybir.ActivationFunctionType.Square,
                    accum_out=out_sb[:, 588 + c:589 + c],
                ).then_inc(s_eng["act"], 1)

        @block.vector
        def _(vector):
            vector.memset(zt[:], 0.0).then_inc(s_z, 1)
            for c in range(NCH):
                if sq_assign.get(c) != "dve":
                    continue
                vector.wait_ge(s_er, th_piece[piece_of_chunk[c]])
                k = sq_k[c]
                vector.scalar_tensor_tensor(
                    out=scr_dve[:, 0:k, :],
                    in0=ersb[:, c * CH:c * CH + k, :], scalar=0.0,
                    in1=ersb[:, c * CH:c * CH + k, :],
                    op0=mybir.AluOpType.bypass, op1=mybir.AluOpType.mult,
                    accum_out=out_sb[:, 588 + c:589 + c],
                ).then_inc(s_eng["dve"], 1)
            for j, (lo, hi) in enumerate(((0, 196), (196, 392), (392, 588))):
                vector.wait_ge(s_pes[j], 1)
                npart = 96 if j < 2 else 4
                vector.tensor_copy(out_sb[0:npart, lo:hi], ps[j][:]
                                   ).then_inc(s_cp, 1)

    _nc_cache[key] = nc
    return nc


# ---------------------------------------------------------------- entry point
LAST = {}


def kernel(os, h, t, v):
    h = np.asarray(h)
    t = np.asarray(t)
    v = np.asarray(v)
    ER, WT, meta, hostD, termB_host, n1 = _host_stage(h, t, v)
    nc = _build_nc(meta)
    in_maps = [{"ER": ER[i], "WT": WT[i]} for i in range(NCORES)]
    res = run_bass_kernel_spmd(
        nc, in_maps, list(range(NCORES)),
        trace=bool(_os.environ.get("KERNEL_TRACE")),
    )
    LAST["res"] = res

    M1p = _tables()[0]
    sq_cols = [588 + c for c, _ in meta["sq_k"]]
    termA = 0.0
    termB_scatter = 0.0
    for i in range(NCORES):
        out = res.results[i]["OUT"].astype(np.float64)
        termA += float(out[:, sq_cols].sum())
        SU = np.concatenate(
            [out[0:96, 0:196], out[0:96, 196:392], out[0:4, 392:588]], axis=0)
        termB_scatter += float((SU * M1p).sum())
    total = termA - 2.0 * (termB_scatter + termB_host) + hostD
    return np.float32(total / n1)
